# revision 24
# baseline (speedup 1.0000x reference)
"""Trainium2 Bass kernel for the Masker module.

Pipeline (per NeuronCore c of 8, model-parallel MLP + batch-sharded
Gumbel-softmax loop):

  L1: h1 = relu(BN(f @ w1.T))      -- w1 sharded by output feature (1024/core),
                                      BN is per-feature and features are local.
  L2: h2 = relu(BN(h1 @ w2.T))     -- w2 sharded by input feature; partial
                                      [8192,128] products reduce-scattered so
                                      each core owns 1024 output features.
  L3: logits = BN(h2 @ w3.T)       -- w3 sharded by input feature; partials
                                      [128,2048] reduce-scattered over batch so
                                      each core owns its 16 rows; BN stats
                                      all-reduced ([16,256] packed sums).
  Loop: 1024 sequential soft gumbel-softmax steps on this core's 16 rows,
        laid out as [128, 256] (row-major: partition p = 8*r + j holds row r,
        columns 256j..256j+255, so the per-step noise DMA is one contiguous
        128KB read). Row sums are formed per-partition by the ScalarE exp's
        accum_out, then summed/broadcast across each row's 8 partitions with
        one 128x128 matmul (S[p,q] = p//8==q//8). The state is kept
        unnormalized: e (exps) and r (1/rowsum); the fused DVE op
        scalar_tensor_tensor gives x = e*r + g and z = max(e*r, z) in one
        instruction each. The z-update for step t-1 is emitted after step
        t's x so it fills the DVE gap while ScalarE runs the exp instead of
        delaying the recip -> x dependency chain.

The gumbel noise depends only on the fixed key(42), not on the inputs, so it
is generated host-side with jax (CPU backend) once and streamed to each core
(128 MB/core, fp32).

Biases b1/b2/b3 are dropped: BatchNorm subtracts the per-feature mean, so
additive per-feature constants cancel exactly.
"""

import sys

for _p in ("/opt/trn_rl_repo",):
    if _p not in sys.path:
        sys.path.insert(0, _p)

import numpy as np

import concourse.bacc as bacc
import concourse.mybir as mybir
import concourse.tile as tile
from concourse import bass_utils

F32 = mybir.dt.float32
F32R = mybir.dt.float32r
AF = mybir.ActivationFunctionType
OP = mybir.AluOpType

N_CORES = 8
B, D, M, C = 128, 2048, 8192, 2048
MC = M // N_CORES  # features per core (1024)
RB = B // N_CORES  # batch rows per core (16)
K_STEPS = 1024
TAU = 0.5
EPS = 1e-5
NJ = C // 256  # 8 column chunks per row


def build_kernel(k_steps=None, single_core=False):
    if k_steps is None:
        k_steps = K_STEPS
    nc = bacc.Bacc("TRN2", target_bir_lowering=False, debug=False,
                   num_devices=1 if single_core else N_CORES)

    fT = nc.dram_tensor("fT", [D, B], F32, kind="ExternalInput")
    w1T = nc.dram_tensor("w1T", [D, MC], F32, kind="ExternalInput")
    w2T = nc.dram_tensor("w2T", [MC, M], F32, kind="ExternalInput")
    w3T = nc.dram_tensor("w3T", [MC, C], F32, kind="ExternalInput")
    g1v = nc.dram_tensor("g1v", [128, 8], F32, kind="ExternalInput")
    be1v = nc.dram_tensor("be1v", [128, 8], F32, kind="ExternalInput")
    g2v = nc.dram_tensor("g2v", [128, 8], F32, kind="ExternalInput")
    be2v = nc.dram_tensor("be2v", [128, 8], F32, kind="ExternalInput")
    Sfull = nc.dram_tensor("Sfull", [128, 128], F32, kind="ExternalInput")
    Gmat = nc.dram_tensor("Gmat", [128, 8], F32, kind="ExternalInput")
    GmatT = nc.dram_tensor("GmatT", [8, 128], F32, kind="ExternalInput")
    noise = nc.dram_tensor("noise", [max(k_steps, 1), RB, C], F32, kind="ExternalInput")
    zout = nc.dram_tensor("zout", [RB, C], F32, kind="ExternalOutput")

    groups = [list(range(N_CORES))]

    with tile.TileContext(nc) as tc:
        with (
            tc.tile_pool(name="dram", bufs=1, space="DRAM") as dram,
            tc.tile_pool(name="consts", bufs=1) as consts,
            tc.tile_pool(name="acts", bufs=1) as acts,
            tc.tile_pool(name="stats", bufs=1) as stats,
        ):
            g1v_sb = consts.tile([128, 8], F32)
            nc.sync.dma_start(g1v_sb[:], g1v[:, :])
            be1v_sb = consts.tile([128, 8], F32)
            nc.sync.dma_start(be1v_sb[:], be1v[:, :])
            g2v_sb = consts.tile([128, 8], F32)
            nc.sync.dma_start(g2v_sb[:], g2v[:, :])
            be2v_sb = consts.tile([128, 8], F32)
            nc.sync.dma_start(be2v_sb[:], be2v[:, :])

            # ---------------- BN helper (feature-on-partition layout) ------
            def bn_relu(ps_tile, nb, gam, bet, out_ap, func):
                """out = func(scale * x + shift) with batch stats over free dim.

                ps_tile: [128, nb] pre-BN values (feature rows, batch cols).
                gam/bet: [128, 1] affine params (or None).
                """
                s1 = stats.tile([128, 1], F32, tag="bn_s1", bufs=4)
                nc.vector.tensor_reduce(s1[:], ps_tile, axis=mybir.AxisListType.X,
                                        op=OP.add)
                sqs = acts.tile([128, nb], F32, tag="bn_sqs", bufs=2)
                ssq = stats.tile([128, 1], F32, tag="bn_ssq", bufs=4)
                nc.scalar.activation(sqs[:], ps_tile, AF.Square, accum_out=ssq[:])
                mean = stats.tile([128, 1], F32, tag="bn_mean", bufs=4)
                nc.vector.tensor_scalar_mul(mean[:], s1[:], 1.0 / nb)
                ex2 = stats.tile([128, 1], F32, tag="bn_ex2", bufs=4)
                # ex2 = ssq/nb + EPS (eps folded in for the sqrt)
                nc.vector.tensor_scalar(ex2[:], ssq[:], 1.0 / nb, EPS,
                                        op0=OP.mult, op1=OP.add)
                var = stats.tile([128, 1], F32, tag="bn_var", bufs=4)
                # var = ex2 - mean*mean  ==  (mean * -mean) + ex2
                nmean = stats.tile([128, 1], F32, tag="bn_nmean", bufs=4)
                nc.vector.tensor_scalar_mul(nmean[:], mean[:], -1.0)
                nc.vector.scalar_tensor_tensor(var[:], mean[:], nmean[:], ex2[:],
                                               op0=OP.mult, op1=OP.add)
                std = stats.tile([128, 1], F32, tag="bn_std", bufs=4)
                nc.scalar.activation(std[:], var[:], AF.Sqrt)
                rstd = stats.tile([128, 1], F32, tag="bn_rstd", bufs=4)
                nc.vector.reciprocal(rstd[:], std[:])
                scale = stats.tile([128, 1], F32, tag="bn_scale", bufs=4)
                if gam is not None:
                    nc.vector.tensor_mul(scale[:], rstd[:], gam)
                else:
                    nc.vector.tensor_copy(scale[:], rstd[:])
                # shift = bet - mean*scale
                shift = stats.tile([128, 1], F32, tag="bn_shift", bufs=4)
                if bet is not None:
                    nc.vector.scalar_tensor_tensor(shift[:], nmean[:], scale[:],
                                                   bet, op0=OP.mult, op1=OP.add)
                else:
                    nc.vector.tensor_mul(shift[:], nmean[:], scale[:])
                nc.scalar.activation(out_ap, ps_tile, func,
                                     bias=shift[:], scale=scale[:])

            # ---------------- L1: h1 = relu(BN(f @ w1.T)) ------------------
            h1 = acts.tile([128, 8 * 128], F32)  # [m1 within tile, kt*128 + b]
            with (
                tc.tile_pool(name="l1", bufs=1) as l1p,
                tc.tile_pool(name="l1ps", bufs=2, space="PSUM") as l1ps,
            ):
                fT_sb = l1p.tile([128, 16 * 128], F32)
                nc.sync.dma_start(fT_sb[:],
                                  fT[:, :].rearrange("(kt p) b -> p kt b", p=128))
                w1_sb = l1p.tile([128, 16 * MC], F32)
                nc.sync.dma_start(w1_sb[:],
                                  w1T[:, :].rearrange("(kt p) m -> p kt m", p=128))
                for mt in range(8):
                    ps1 = l1ps.tile([128, 128], F32, tag="l1ps")
                    for kt in range(16):
                        nc.tensor.matmul(
                            ps1[:],
                            w1_sb[:, kt * MC + mt * 128:
                                  kt * MC + (mt + 1) * 128],
                            fT_sb[:, kt * 128:(kt + 1) * 128],
                            start=(kt == 0), stop=(kt == 15),
                        )
                    bn_relu(ps1[:], 128, g1v_sb[:, mt:mt + 1], be1v_sb[:, mt:mt + 1],
                            h1[:, mt * 128:(mt + 1) * 128], AF.Relu)

            # ---------------- L2: partial h2 = h1 @ w2.T -------------------
            h2part = dram.tile([M, B], F32)
            with (
                tc.tile_pool(name="l2w", bufs=3) as l2w,
                tc.tile_pool(name="l2ps", bufs=4, space="PSUM") as l2ps,
            ):
                for mt in range(M // 128):
                    wt = l2w.tile([128, 8 * 128], F32, tag="l2wt")
                    nc.sync.dma_start(
                        wt[:],
                        w2T[:, mt * 128:(mt + 1) * 128]
                        .rearrange("(kt p) m -> p kt m", p=128))
                    ps2 = l2ps.tile([128, 128], F32, tag="l2ps")
                    for kt in range(8):
                        nc.tensor.matmul(
                            ps2[:],
                            wt[:, kt * 128:(kt + 1) * 128],
                            h1[:, kt * 128:(kt + 1) * 128],
                            start=(kt == 0), stop=(kt == 7),
                        )
                    h2c = l2w.tile([128, 128], F32, tag="l2sb", bufs=3)
                    nc.scalar.copy(h2c[:], ps2[:])
                    nc.sync.dma_start(h2part[mt * 128:(mt + 1) * 128, :], h2c[:])

            # reduce-scatter over cores: each core keeps its 1024 features
            h2rs = dram.tile([MC, B], F32)
            if single_core:
                nc.sync.dma_start(h2rs[:], h2part[0:MC, :])
            else:
                nc.gpsimd.collective_compute(
                    "ReduceScatter", OP.add, replica_groups=groups,
                    ins=[h2part[:]], outs=[h2rs[:]])

            # ---------------- L2 BN + relu ---------------------------------
            h2 = acts.tile([128, 8 * 128], F32)
            with tc.tile_pool(name="l2bn", bufs=1) as l2bn:
                h2pre = l2bn.tile([128, 8 * 128], F32)
                nc.sync.dma_start(h2pre[:],
                                  h2rs[:, :].rearrange("(kt p) b -> p kt b", p=128))
                for kt in range(8):
                    bn_relu(h2pre[:, kt * 128:(kt + 1) * 128], 128,
                            g2v_sb[:, kt:kt + 1], be2v_sb[:, kt:kt + 1],
                            h2[:, kt * 128:(kt + 1) * 128], AF.Relu)

            # ---------------- L3: partial logits = h2 @ w3.T ---------------
            l3part = dram.tile([B, C], F32)
            with (
                tc.tile_pool(name="l3w", bufs=1) as l3w,
                tc.tile_pool(name="l3ps", bufs=4, space="PSUM") as l3ps,
            ):
                w3_sb = l3w.tile([128, 8 * C], F32)
                nc.sync.dma_start(w3_sb[:],
                                  w3T[:, :].rearrange("(kt p) m -> p kt m", p=128))
                for nt in range(4):
                    ps3 = l3ps.tile([128, 512], F32, tag="l3ps")
                    for kt in range(8):
                        nc.tensor.matmul(
                            ps3[:],
                            h2[:, kt * 128:(kt + 1) * 128],
                            w3_sb[:, kt * C + nt * 512:
                                  kt * C + (nt + 1) * 512],
                            start=(kt == 0), stop=(kt == 7),
                        )
                    l3c = l3w.tile([128, 512], F32, tag="l3sb", bufs=2)
                    nc.scalar.copy(l3c[:], ps3[:])
                    nc.sync.dma_start(l3part[:, nt * 512:(nt + 1) * 512], l3c[:])

            # reduce-scatter over batch rows: core keeps its 16 rows (summed)
            l3rs = dram.tile([RB, C], F32)
            if single_core:
                nc.sync.dma_start(l3rs[:], l3part[0:RB, :])
            else:
                nc.gpsimd.collective_compute(
                    "ReduceScatter", OP.add, replica_groups=groups,
                    ins=[l3part[:]], outs=[l3rs[:]])

            # ---------------- the gumbel-softmax loop ----------------------
            with (
                tc.tile_pool(name="loop", bufs=1) as loop,
                tc.tile_pool(name="gio", bufs=4) as gio,
                tc.tile_pool(name="lps", bufs=2, space="PSUM") as lps,
                tc.tile_pool(name="xe", bufs=2) as xe,
            ):
                S_sb = loop.tile([128, 128], F32)
                nc.sync.dma_start(S_sb[:], Sfull[:, :])
                # state: e (unnormalized softmax numerators), r (1/rowsum),
                # z (running max). Initial "mask" is the BN'd logits: e=m0, r=1.
                e_t = loop.tile([128, 256], F32)
                r_t = loop.tile([128, 1], F32)
                nc.vector.memset(r_t[:], 1.0)
                z_t = loop.tile([128, 256], F32)
                nc.vector.memset(z_t[:], 0.0)

                # ---------- logits BN (affine=False) in loop layout --------
                # m0_raw[p=(16j+r), f] = logits[16c+r, 256j+f]
                with tc.tile_pool(name="statps", bufs=1, space="PSUM") as statps:
                    m0_raw = consts.tile([128, 256], F32)
                    nc.sync.dma_start(
                        m0_raw[:],
                        l3rs[:, :].rearrange("r (j f) -> r j f", j=NJ))
                    Gm = consts.tile([128, 8], F32)
                    nc.sync.dma_start(Gm[:], Gmat[:, :])
                    GmT = consts.tile([8, 128], F32)
                    nc.sync.dma_start(GmT[:], GmatT[:, :])

                    # per-(chunk, col) partial sums over this core's 16 rows
                    sq0 = acts.tile([128, 256], F32, tag="bn_sqs", bufs=2)
                    nc.scalar.activation(sq0[:], m0_raw[:], AF.Square)
                    ps_s = statps.tile([8, 256], F32, tag="ps_s")
                    nc.tensor.matmul(ps_s[:], Gm[:], m0_raw[:], start=True,
                                     stop=True)
                    ps_q = statps.tile([8, 256], F32, tag="ps_q")
                    nc.tensor.matmul(ps_q[:], Gm[:], sq0[:], start=True,
                                     stop=True)
                    stat_sb = stats.tile([8, 512], F32)
                    nc.vector.tensor_copy(stat_sb[:, 0:256], ps_s[:])
                    nc.vector.tensor_copy(stat_sb[:, 256:512], ps_q[:])
                    stat_in = dram.tile([8, 512], F32)
                    nc.sync.dma_start(stat_in[:], stat_sb[:])
                    stat_out = dram.tile([8, 512], F32, addr_space="Shared")
                    if single_core:
                        nc.sync.dma_start(stat_out[:], stat_in[:])
                    else:
                        nc.gpsimd.collective_compute(
                            "AllReduce", OP.add, replica_groups=groups,
                            ins=[stat_in[:]], outs=[stat_out[:]])
                    stat_fl = stats.tile([8, 512], F32)
                    nc.sync.dma_start(stat_fl[:], stat_out[:])

                    meanc = stats.tile([8, 256], F32)
                    nc.vector.tensor_scalar_mul(meanc[:], stat_fl[:, 0:256],
                                                1.0 / B)
                    ex2c = stats.tile([8, 256], F32)
                    nc.vector.tensor_scalar(ex2c[:], stat_fl[:, 256:512],
                                            1.0 / B, EPS,
                                            op0=OP.mult, op1=OP.add)
                    nmeanc = stats.tile([8, 256], F32)
                    nc.vector.tensor_scalar_mul(nmeanc[:], meanc[:], -1.0)
                    varc = stats.tile([8, 256], F32)
                    nc.vector.tensor_tensor(varc[:], meanc[:], nmeanc[:],
                                            op=OP.mult)
                    nc.vector.tensor_tensor(varc[:], varc[:], ex2c[:],
                                            op=OP.add)
                    stdc = stats.tile([8, 256], F32)
                    nc.scalar.activation(stdc[:], varc[:], AF.Sqrt)
                    rstdc = stats.tile([8, 256], F32)
                    nc.vector.reciprocal(rstdc[:], stdc[:])
                    shiftc = stats.tile([8, 256], F32)
                    nc.vector.tensor_tensor(shiftc[:], nmeanc[:], rstdc[:],
                                            op=OP.mult)

                    # broadcast to [128, 256] via GmatT matmuls
                    ps_sc = statps.tile([128, 256], F32, tag="ps_sc")
                    nc.tensor.matmul(ps_sc[:], GmT[:], rstdc[:], start=True,
                                     stop=True)
                    ps_sh = statps.tile([128, 256], F32, tag="ps_sh")
                    nc.tensor.matmul(ps_sh[:], GmT[:], shiftc[:], start=True,
                                     stop=True)
                    nc.vector.tensor_tensor(e_t[:], m0_raw[:], ps_sc[:],
                                            op=OP.mult)
                    nc.vector.tensor_tensor(e_t[:], e_t[:], ps_sh[:],
                                            op=OP.add)

                if k_steps == 0:
                    nc.vector.tensor_copy(z_t[:], e_t[:])

                for t in range(k_steps):
                    g_t = gio.tile([128, 256], F32, tag="g", bufs=4)
                    nc.sync.dma_start(
                        g_t[:],
                        noise[t, :, :].rearrange("r (j f) -> r j f", j=NJ))
                    # x = e*r + g   (mask + gumbel)
                    x_t = xe.tile([128, 256], F32, tag="x", bufs=2)
                    nc.vector.scalar_tensor_tensor(
                        x_t[:], e_t[:], r_t[:], g_t[:], op0=OP.mult, op1=OP.add)
                    # z-update for the PREVIOUS step, emitted after this
                    # step's stt so the DVE chain (recip -> stt_x) stays
                    # tight; the z op fills the gap while ACT runs exp.
                    if t > 0:
                        nc.vector.scalar_tensor_tensor(
                            z_t[:], e_t[:], r_t[:], z_t[:],
                            op0=OP.mult, op1=OP.max)
                    # e' = exp(x / tau), s = per-partition sums
                    e_n = loop.tile([128, 256], F32, tag="e_n", bufs=3)
                    s_t = stats.tile([128, 1], F32, tag="s_t", bufs=3)
                    nc.scalar.activation(e_n[:], x_t[:], AF.Exp,
                                         scale=1.0 / TAU, accum_out=s_t[:])
                    # rowsum broadcast: R[p] = sum over row-partners of s
                    R_t = lps.tile([128, 1], F32, tag="R", bufs=2)
                    nc.tensor.matmul(R_t[:], S_sb[:], s_t[:], start=True, stop=True)
                    r_n = stats.tile([128, 1], F32, tag="r_n", bufs=3)
                    nc.vector.reciprocal(r_n[:], R_t[:])
                    e_t, r_t = e_n, r_n
                # final step's z-update
                nc.vector.scalar_tensor_tensor(
                    z_t[:], e_t[:], r_t[:], z_t[:], op0=OP.mult, op1=OP.max)

                nc.sync.dma_start(
                    zout[:, :].rearrange("r (j f) -> r j f", j=NJ), z_t[:])

    nc.compile()
    return nc


_CACHE = {}


def _host_prep(f, w1, g1, be1, w2, g2, be2, w3):
    """Build the per-core input maps (everything except fT is cacheable)."""
    S = (np.arange(128)[:, None] // 8 == np.arange(128)[None, :] // 8)
    Sfull = S.astype(np.float32)
    G = (np.arange(128)[:, None] % 8 == np.arange(8)[None, :])
    Gmat = G.astype(np.float32)
    GmatT = np.ascontiguousarray(Gmat.T)

    noise = _gen_noise()

    in_maps = []
    fT = np.ascontiguousarray(np.asarray(f, np.float32).T)
    for c in range(N_CORES):
        sl = slice(MC * c, MC * (c + 1))
        m = {
            "fT": fT,
            "w1T": np.ascontiguousarray(np.asarray(w1[sl, :], np.float32).T),
            "w2T": np.ascontiguousarray(np.asarray(w2[:, sl], np.float32).T),
            "w3T": np.ascontiguousarray(np.asarray(w3[:, sl], np.float32).T),
            "g1v": np.ascontiguousarray(
                np.asarray(g1[sl], np.float32).reshape(8, 128).T),
            "be1v": np.ascontiguousarray(
                np.asarray(be1[sl], np.float32).reshape(8, 128).T),
            "g2v": np.ascontiguousarray(
                np.asarray(g2[sl], np.float32).reshape(8, 128).T),
            "be2v": np.ascontiguousarray(
                np.asarray(be2[sl], np.float32).reshape(8, 128).T),
            "Sfull": Sfull,
            "Gmat": Gmat,
            "GmatT": GmatT,
            "noise": np.ascontiguousarray(noise[:, RB * c:RB * (c + 1), :]),
        }
        in_maps.append(m)
    return in_maps


def _gen_noise():
    if "noise" in _CACHE:
        return _CACHE["noise"]
    import jax
    import jax.numpy as jnp

    cpu = jax.devices("cpu")[0]

    @jax.jit
    def gen(rng):
        # mirror the reference's scan exactly (gumbel inside the scan):
        # vmap(gumbel) over pre-split keys gives DIFFERENT bits.
        def step(k, _):
            k, sub = jax.random.split(k)
            return k, jax.random.gumbel(sub, (B, C), jnp.float32)

        _, g = jax.lax.scan(step, rng, None, length=max(K_STEPS, 1))
        return g

    with jax.default_device(cpu):
        g = np.asarray(gen(jax.random.key(42)))
    _CACHE["noise"] = g
    return g


def kernel(f, w1, b1, g1, be1, w2, b2, g2, be2, w3, b3):
    if "nc" not in _CACHE:
        _CACHE["nc"] = build_kernel()
    nc = _CACHE["nc"]
    in_maps = _host_prep(f, w1, g1, be1, w2, g2, be2, w3)
    res = bass_utils.run_bass_kernel_spmd(nc, in_maps,
                                          core_ids=list(range(N_CORES)))
    z = np.concatenate([res.results[c]["zout"] for c in range(N_CORES)], axis=0)
    return z


# revision 25
# speedup vs baseline: 1.0563x; 1.0563x over previous
"""Trainium2 Bass kernel for the Masker module.

Pipeline (per NeuronCore c of 8, model-parallel MLP + batch-sharded
Gumbel-softmax loop):

  L1: h1 = relu(BN(f @ w1.T))      -- w1 sharded by output feature (1024/core),
                                      BN is per-feature and features are local.
  L2: h2 = relu(BN(h1 @ w2.T))     -- w2 sharded by input feature; partial
                                      [8192,128] products reduce-scattered so
                                      each core owns 1024 output features.
  L3: logits = BN(h2 @ w3.T)       -- w3 sharded by input feature; partials
                                      [128,2048] reduce-scattered over batch so
                                      each core owns its 16 rows; BN stats
                                      all-reduced ([16,256] packed sums).
  Loop: 1024 sequential soft gumbel-softmax steps on this core's 16 rows,
        laid out as [128, 256] (row-major: partition p = 8*r + j holds row r,
        columns 256j..256j+255, so the per-step noise DMA is one contiguous
        128KB read). Row sums are formed per-partition by the ScalarE exp's
        accum_out, then summed/broadcast across each row's 8 partitions with
        one 128x128 matmul (S[p,q] = p//8==q//8). The state is kept
        unnormalized: e (exps) and r (1/rowsum); the fused DVE op
        scalar_tensor_tensor gives x = e*r + g and z = max(e*r, z) in one
        instruction each. The z-update for step t-1 is emitted after step
        t's x so it fills the DVE gap while ScalarE runs the exp instead of
        delaying the recip -> x dependency chain.

The gumbel noise depends only on the fixed key(42), not on the inputs, so it
is generated host-side with jax (CPU backend) once and streamed to each core
(128 MB/core, fp32).

Biases b1/b2/b3 are dropped: BatchNorm subtracts the per-feature mean, so
additive per-feature constants cancel exactly.
"""

import sys

for _p in ("/opt/trn_rl_repo",):
    if _p not in sys.path:
        sys.path.insert(0, _p)

import numpy as np

import concourse.bacc as bacc
import concourse.mybir as mybir
import concourse.tile as tile
from concourse import bass_utils

F32 = mybir.dt.float32
F32R = mybir.dt.float32r
AF = mybir.ActivationFunctionType
OP = mybir.AluOpType

N_CORES = 8
B, D, M, C = 128, 2048, 8192, 2048
MC = M // N_CORES  # features per core (1024)
RB = B // N_CORES  # batch rows per core (16)
K_STEPS = 1024
TAU = 0.5
EPS = 1e-5
NJ = C // 256  # 8 column chunks per row


def build_kernel(k_steps=None, single_core=False):
    if k_steps is None:
        k_steps = K_STEPS
    nc = bacc.Bacc("TRN2", target_bir_lowering=False, debug=False,
                   num_devices=1 if single_core else N_CORES)

    fT = nc.dram_tensor("fT", [D, B], F32, kind="ExternalInput")
    w1T = nc.dram_tensor("w1T", [D, MC], F32, kind="ExternalInput")
    w2T = nc.dram_tensor("w2T", [MC, M], F32, kind="ExternalInput")
    w3T = nc.dram_tensor("w3T", [MC, C], F32, kind="ExternalInput")
    g1v = nc.dram_tensor("g1v", [128, 8], F32, kind="ExternalInput")
    be1v = nc.dram_tensor("be1v", [128, 8], F32, kind="ExternalInput")
    g2v = nc.dram_tensor("g2v", [128, 8], F32, kind="ExternalInput")
    be2v = nc.dram_tensor("be2v", [128, 8], F32, kind="ExternalInput")
    Sfull = nc.dram_tensor("Sfull", [128, 128], F32, kind="ExternalInput")
    Gmat = nc.dram_tensor("Gmat", [128, 8], F32, kind="ExternalInput")
    GmatT = nc.dram_tensor("GmatT", [8, 128], F32, kind="ExternalInput")
    noise = nc.dram_tensor("noise", [max(k_steps, 1), RB, C], F32, kind="ExternalInput")
    zout = nc.dram_tensor("zout", [RB, C], F32, kind="ExternalOutput")

    groups = [list(range(N_CORES))]

    with tile.TileContext(nc) as tc:
        with (
            tc.tile_pool(name="dram", bufs=1, space="DRAM") as dram,
            tc.tile_pool(name="consts", bufs=1) as consts,
            tc.tile_pool(name="acts", bufs=1) as acts,
            tc.tile_pool(name="stats", bufs=1) as stats,
        ):
            g1v_sb = consts.tile([128, 8], F32)
            nc.sync.dma_start(g1v_sb[:], g1v[:, :])
            be1v_sb = consts.tile([128, 8], F32)
            nc.sync.dma_start(be1v_sb[:], be1v[:, :])
            g2v_sb = consts.tile([128, 8], F32)
            nc.sync.dma_start(g2v_sb[:], g2v[:, :])
            be2v_sb = consts.tile([128, 8], F32)
            nc.sync.dma_start(be2v_sb[:], be2v[:, :])

            # ---------------- BN helper (feature-on-partition layout) ------
            def bn_relu(ps_tile, nb, gam, bet, out_ap, func):
                """out = func(scale * x + shift) with batch stats over free dim.

                ps_tile: [128, nb] pre-BN values (feature rows, batch cols).
                gam/bet: [128, 1] affine params (or None).
                """
                s1 = stats.tile([128, 1], F32, tag="bn_s1", bufs=4)
                nc.vector.tensor_reduce(s1[:], ps_tile, axis=mybir.AxisListType.X,
                                        op=OP.add)
                sqs = acts.tile([128, nb], F32, tag="bn_sqs", bufs=2)
                ssq = stats.tile([128, 1], F32, tag="bn_ssq", bufs=4)
                nc.scalar.activation(sqs[:], ps_tile, AF.Square, accum_out=ssq[:])
                mean = stats.tile([128, 1], F32, tag="bn_mean", bufs=4)
                nc.vector.tensor_scalar_mul(mean[:], s1[:], 1.0 / nb)
                ex2 = stats.tile([128, 1], F32, tag="bn_ex2", bufs=4)
                # ex2 = ssq/nb + EPS (eps folded in for the sqrt)
                nc.vector.tensor_scalar(ex2[:], ssq[:], 1.0 / nb, EPS,
                                        op0=OP.mult, op1=OP.add)
                var = stats.tile([128, 1], F32, tag="bn_var", bufs=4)
                # var = ex2 - mean*mean  ==  (mean * -mean) + ex2
                nmean = stats.tile([128, 1], F32, tag="bn_nmean", bufs=4)
                nc.vector.tensor_scalar_mul(nmean[:], mean[:], -1.0)
                nc.vector.scalar_tensor_tensor(var[:], mean[:], nmean[:], ex2[:],
                                               op0=OP.mult, op1=OP.add)
                std = stats.tile([128, 1], F32, tag="bn_std", bufs=4)
                nc.scalar.activation(std[:], var[:], AF.Sqrt)
                rstd = stats.tile([128, 1], F32, tag="bn_rstd", bufs=4)
                nc.vector.reciprocal(rstd[:], std[:])
                scale = stats.tile([128, 1], F32, tag="bn_scale", bufs=4)
                if gam is not None:
                    nc.vector.tensor_mul(scale[:], rstd[:], gam)
                else:
                    nc.vector.tensor_copy(scale[:], rstd[:])
                # shift = bet - mean*scale
                shift = stats.tile([128, 1], F32, tag="bn_shift", bufs=4)
                if bet is not None:
                    nc.vector.scalar_tensor_tensor(shift[:], nmean[:], scale[:],
                                                   bet, op0=OP.mult, op1=OP.add)
                else:
                    nc.vector.tensor_mul(shift[:], nmean[:], scale[:])
                nc.scalar.activation(out_ap, ps_tile, func,
                                     bias=shift[:], scale=scale[:])

            # ---------------- L1: h1 = relu(BN(f @ w1.T)) ------------------
            h1 = acts.tile([128, 8 * 128], F32)  # [m1 within tile, kt*128 + b]
            with (
                tc.tile_pool(name="l1", bufs=1) as l1p,
                tc.tile_pool(name="l1ps", bufs=2, space="PSUM") as l1ps,
            ):
                fT_sb = l1p.tile([128, 16 * 128], F32)
                nc.sync.dma_start(fT_sb[:],
                                  fT[:, :].rearrange("(kt p) b -> p kt b", p=128))
                w1_sb = l1p.tile([128, 16 * MC], F32)
                nc.sync.dma_start(w1_sb[:],
                                  w1T[:, :].rearrange("(kt p) m -> p kt m", p=128))
                for mt in range(8):
                    ps1 = l1ps.tile([128, 128], F32, tag="l1ps")
                    for kt in range(16):
                        nc.tensor.matmul(
                            ps1[:],
                            w1_sb[:, kt * MC + mt * 128:
                                  kt * MC + (mt + 1) * 128],
                            fT_sb[:, kt * 128:(kt + 1) * 128],
                            start=(kt == 0), stop=(kt == 15),
                        )
                    bn_relu(ps1[:], 128, g1v_sb[:, mt:mt + 1], be1v_sb[:, mt:mt + 1],
                            h1[:, mt * 128:(mt + 1) * 128], AF.Relu)

            # ---------------- L2: partial h2 = h1 @ w2.T -------------------
            h2part = dram.tile([M, B], F32)
            with (
                tc.tile_pool(name="l2w", bufs=3) as l2w,
                tc.tile_pool(name="l2ps", bufs=4, space="PSUM") as l2ps,
            ):
                for mt in range(M // 128):
                    wt = l2w.tile([128, 8 * 128], F32, tag="l2wt")
                    nc.sync.dma_start(
                        wt[:],
                        w2T[:, mt * 128:(mt + 1) * 128]
                        .rearrange("(kt p) m -> p kt m", p=128))
                    ps2 = l2ps.tile([128, 128], F32, tag="l2ps")
                    for kt in range(8):
                        nc.tensor.matmul(
                            ps2[:],
                            wt[:, kt * 128:(kt + 1) * 128],
                            h1[:, kt * 128:(kt + 1) * 128],
                            start=(kt == 0), stop=(kt == 7),
                        )
                    h2c = l2w.tile([128, 128], F32, tag="l2sb", bufs=3)
                    nc.scalar.copy(h2c[:], ps2[:])
                    nc.sync.dma_start(h2part[mt * 128:(mt + 1) * 128, :], h2c[:])

            # reduce-scatter over cores: each core keeps its 1024 features
            h2rs = dram.tile([MC, B], F32)
            if single_core:
                nc.sync.dma_start(h2rs[:], h2part[0:MC, :])
            else:
                nc.gpsimd.collective_compute(
                    "ReduceScatter", OP.add, replica_groups=groups,
                    ins=[h2part[:]], outs=[h2rs[:]])

            # ---------------- L2 BN + relu ---------------------------------
            h2 = acts.tile([128, 8 * 128], F32)
            with tc.tile_pool(name="l2bn", bufs=1) as l2bn:
                h2pre = l2bn.tile([128, 8 * 128], F32)
                nc.sync.dma_start(h2pre[:],
                                  h2rs[:, :].rearrange("(kt p) b -> p kt b", p=128))
                for kt in range(8):
                    bn_relu(h2pre[:, kt * 128:(kt + 1) * 128], 128,
                            g2v_sb[:, kt:kt + 1], be2v_sb[:, kt:kt + 1],
                            h2[:, kt * 128:(kt + 1) * 128], AF.Relu)

            # ---------------- L3: partial logits = h2 @ w3.T ---------------
            l3part = dram.tile([B, C], F32)
            with (
                tc.tile_pool(name="l3w", bufs=1) as l3w,
                tc.tile_pool(name="l3ps", bufs=4, space="PSUM") as l3ps,
            ):
                w3_sb = l3w.tile([128, 8 * C], F32)
                nc.sync.dma_start(w3_sb[:],
                                  w3T[:, :].rearrange("(kt p) m -> p kt m", p=128))
                for nt in range(4):
                    ps3 = l3ps.tile([128, 512], F32, tag="l3ps")
                    for kt in range(8):
                        nc.tensor.matmul(
                            ps3[:],
                            h2[:, kt * 128:(kt + 1) * 128],
                            w3_sb[:, kt * C + nt * 512:
                                  kt * C + (nt + 1) * 512],
                            start=(kt == 0), stop=(kt == 7),
                        )
                    l3c = l3w.tile([128, 512], F32, tag="l3sb", bufs=2)
                    nc.scalar.copy(l3c[:], ps3[:])
                    nc.sync.dma_start(l3part[:, nt * 512:(nt + 1) * 512], l3c[:])

            # reduce-scatter over batch rows: core keeps its 16 rows (summed)
            l3rs = dram.tile([RB, C], F32)
            if single_core:
                nc.sync.dma_start(l3rs[:], l3part[0:RB, :])
            else:
                nc.gpsimd.collective_compute(
                    "ReduceScatter", OP.add, replica_groups=groups,
                    ins=[l3part[:]], outs=[l3rs[:]])

            # ---------------- the gumbel-softmax loop ----------------------
            with (
                tc.tile_pool(name="loop", bufs=1) as loop,
                tc.tile_pool(name="gio", bufs=4) as gio,
                tc.tile_pool(name="lps", bufs=2, space="PSUM") as lps,
                tc.tile_pool(name="xe", bufs=2) as xe,
            ):
                S_sb = loop.tile([128, 128], F32)
                nc.sync.dma_start(S_sb[:], Sfull[:, :])
                # state: e (unnormalized softmax numerators), r (1/rowsum),
                # z (running max). Initial "mask" is the BN'd logits: e=m0, r=1.
                e_t = loop.tile([128, 256], F32)
                r_t = loop.tile([128, 1], F32)
                nc.vector.memset(r_t[:], 2.0)
                z_t = loop.tile([128, 256], F32)
                nc.vector.memset(z_t[:], 0.0)

                # ---------- logits BN (affine=False) in loop layout --------
                # m0_raw[p=(16j+r), f] = logits[16c+r, 256j+f]
                with tc.tile_pool(name="statps", bufs=1, space="PSUM") as statps:
                    m0_raw = consts.tile([128, 256], F32)
                    nc.sync.dma_start(
                        m0_raw[:],
                        l3rs[:, :].rearrange("r (j f) -> r j f", j=NJ))
                    Gm = consts.tile([128, 8], F32)
                    nc.sync.dma_start(Gm[:], Gmat[:, :])
                    GmT = consts.tile([8, 128], F32)
                    nc.sync.dma_start(GmT[:], GmatT[:, :])

                    # per-(chunk, col) partial sums over this core's 16 rows
                    sq0 = acts.tile([128, 256], F32, tag="bn_sqs", bufs=2)
                    nc.scalar.activation(sq0[:], m0_raw[:], AF.Square)
                    ps_s = statps.tile([8, 256], F32, tag="ps_s")
                    nc.tensor.matmul(ps_s[:], Gm[:], m0_raw[:], start=True,
                                     stop=True)
                    ps_q = statps.tile([8, 256], F32, tag="ps_q")
                    nc.tensor.matmul(ps_q[:], Gm[:], sq0[:], start=True,
                                     stop=True)
                    stat_sb = stats.tile([8, 512], F32)
                    nc.vector.tensor_copy(stat_sb[:, 0:256], ps_s[:])
                    nc.vector.tensor_copy(stat_sb[:, 256:512], ps_q[:])
                    stat_in = dram.tile([8, 512], F32)
                    nc.sync.dma_start(stat_in[:], stat_sb[:])
                    stat_out = dram.tile([8, 512], F32, addr_space="Shared")
                    if single_core:
                        nc.sync.dma_start(stat_out[:], stat_in[:])
                    else:
                        nc.gpsimd.collective_compute(
                            "AllReduce", OP.add, replica_groups=groups,
                            ins=[stat_in[:]], outs=[stat_out[:]])
                    stat_fl = stats.tile([8, 512], F32)
                    nc.sync.dma_start(stat_fl[:], stat_out[:])

                    meanc = stats.tile([8, 256], F32)
                    nc.vector.tensor_scalar_mul(meanc[:], stat_fl[:, 0:256],
                                                1.0 / B)
                    ex2c = stats.tile([8, 256], F32)
                    nc.vector.tensor_scalar(ex2c[:], stat_fl[:, 256:512],
                                            1.0 / B, EPS,
                                            op0=OP.mult, op1=OP.add)
                    nmeanc = stats.tile([8, 256], F32)
                    nc.vector.tensor_scalar_mul(nmeanc[:], meanc[:], -1.0)
                    varc = stats.tile([8, 256], F32)
                    nc.vector.tensor_tensor(varc[:], meanc[:], nmeanc[:],
                                            op=OP.mult)
                    nc.vector.tensor_tensor(varc[:], varc[:], ex2c[:],
                                            op=OP.add)
                    stdc = stats.tile([8, 256], F32)
                    nc.scalar.activation(stdc[:], varc[:], AF.Sqrt)
                    rstdc = stats.tile([8, 256], F32)
                    nc.vector.reciprocal(rstdc[:], stdc[:])
                    shiftc = stats.tile([8, 256], F32)
                    nc.vector.tensor_tensor(shiftc[:], nmeanc[:], rstdc[:],
                                            op=OP.mult)

                    # broadcast to [128, 256] via GmatT matmuls
                    ps_sc = statps.tile([128, 256], F32, tag="ps_sc")
                    nc.tensor.matmul(ps_sc[:], GmT[:], rstdc[:], start=True,
                                     stop=True)
                    ps_sh = statps.tile([128, 256], F32, tag="ps_sh")
                    nc.tensor.matmul(ps_sh[:], GmT[:], shiftc[:], start=True,
                                     stop=True)
                    nc.vector.tensor_tensor(e_t[:], m0_raw[:], ps_sc[:],
                                            op=OP.mult)
                    nc.vector.tensor_tensor(e_t[:], e_t[:], ps_sh[:],
                                            op=OP.add)

                if k_steps == 0:
                    nc.vector.tensor_copy(z_t[:], e_t[:])

                # Noise arrives as E = exp(2g). Step: u = exp(e * rh)
                # (per-partition scale rh = 2/rowsum via the half-valued S
                # matrix), then one fused DVE op e' = u * E with accum_out
                # giving the per-partition sums -- no ScalarE accumulator
                # read on the chain. z is kept doubled (max of e*rh) and
                # halved once at the end.
                for t in range(k_steps):
                    g_t = gio.tile([128, 256], F32, tag="g", bufs=4)
                    nc.sync.dma_start(
                        g_t[:],
                        noise[t, :, :].rearrange("r (j f) -> r j f", j=NJ))
                    u_t = xe.tile([128, 256], F32, tag="u", bufs=2)
                    nc.scalar.activation(u_t[:], e_t[:], AF.Exp, scale=r_t[:])
                    e_n = loop.tile([128, 256], F32, tag="e_n", bufs=3)
                    s_t = stats.tile([128, 1], F32, tag="s_t", bufs=3)
                    nc.vector.scalar_tensor_tensor(
                        e_n[:], u_t[:], 1.0, g_t[:], op0=OP.mult, op1=OP.mult,
                        accum_out=s_t[:])
                    R_t = lps.tile([128, 1], F32, tag="R", bufs=2)
                    nc.tensor.matmul(R_t[:], S_sb[:], s_t[:], start=True, stop=True)
                    r_n = stats.tile([128, 1], F32, tag="r_n", bufs=3)
                    nc.vector.reciprocal(r_n[:], R_t[:])
                    # z-update for the PREVIOUS step, emitted after the recip
                    # so it fills the DVE gap while ACT runs the next exp.
                    if t > 0:
                        nc.vector.scalar_tensor_tensor(
                            z_t[:], e_t[:], r_t[:], z_t[:],
                            op0=OP.mult, op1=OP.max)
                    e_t, r_t = e_n, r_n
                # final step's z-update, then undo the doubling
                nc.vector.scalar_tensor_tensor(
                    z_t[:], e_t[:], r_t[:], z_t[:], op0=OP.mult, op1=OP.max)
                nc.vector.tensor_scalar_mul(z_t[:], z_t[:], 0.5)

                nc.sync.dma_start(
                    zout[:, :].rearrange("r (j f) -> r j f", j=NJ), z_t[:])

    nc.compile()
    return nc


_CACHE = {}


def _host_prep(f, w1, g1, be1, w2, g2, be2, w3):
    """Build the per-core input maps (everything except fT is cacheable)."""
    S = (np.arange(128)[:, None] // 8 == np.arange(128)[None, :] // 8)
    Sfull = 0.5 * S.astype(np.float32)
    G = (np.arange(128)[:, None] % 8 == np.arange(8)[None, :])
    Gmat = G.astype(np.float32)
    GmatT = np.ascontiguousarray(Gmat.T)

    noise = _gen_noise()

    in_maps = []
    fT = np.ascontiguousarray(np.asarray(f, np.float32).T)
    for c in range(N_CORES):
        sl = slice(MC * c, MC * (c + 1))
        m = {
            "fT": fT,
            "w1T": np.ascontiguousarray(np.asarray(w1[sl, :], np.float32).T),
            "w2T": np.ascontiguousarray(np.asarray(w2[:, sl], np.float32).T),
            "w3T": np.ascontiguousarray(np.asarray(w3[:, sl], np.float32).T),
            "g1v": np.ascontiguousarray(
                np.asarray(g1[sl], np.float32).reshape(8, 128).T),
            "be1v": np.ascontiguousarray(
                np.asarray(be1[sl], np.float32).reshape(8, 128).T),
            "g2v": np.ascontiguousarray(
                np.asarray(g2[sl], np.float32).reshape(8, 128).T),
            "be2v": np.ascontiguousarray(
                np.asarray(be2[sl], np.float32).reshape(8, 128).T),
            "Sfull": Sfull,
            "Gmat": Gmat,
            "GmatT": GmatT,
            "noise": np.ascontiguousarray(noise[:, RB * c:RB * (c + 1), :]),
        }
        in_maps.append(m)
    return in_maps


def _gen_noise():
    if "noise" in _CACHE:
        return _CACHE["noise"]
    import jax
    import jax.numpy as jnp

    cpu = jax.devices("cpu")[0]

    @jax.jit
    def gen(rng):
        # mirror the reference's scan exactly (gumbel inside the scan):
        # vmap(gumbel) over pre-split keys gives DIFFERENT bits.
        def step(k, _):
            k, sub = jax.random.split(k)
            return k, jax.random.gumbel(sub, (B, C), jnp.float32)

        _, g = jax.lax.scan(step, rng, None, length=max(K_STEPS, 1))
        return jnp.exp(2.0 * g)

    with jax.default_device(cpu):
        g = np.asarray(gen(jax.random.key(42)))
    _CACHE["noise"] = g
    return g


def kernel(f, w1, b1, g1, be1, w2, b2, g2, be2, w3, b3):
    if "nc" not in _CACHE:
        _CACHE["nc"] = build_kernel()
    nc = _CACHE["nc"]
    in_maps = _host_prep(f, w1, g1, be1, w2, g2, be2, w3)
    res = bass_utils.run_bass_kernel_spmd(nc, in_maps,
                                          core_ids=list(range(N_CORES)))
    z = np.concatenate([res.results[c]["zout"] for c in range(N_CORES)], axis=0)
    return z
